# revision 5
# baseline (speedup 1.0000x reference)
"""TRN2 Bass kernel for nn_OFTLinear (forward).

Math: the whole OFT chain is linear, so
    out = x @ W_eff + b_eff
with
    W_eff = P_in . BD(R_right) . W^T . BD(R_left) . P_out      [2048 x 2048]
    b_eff = (BD(R_left)^T b)[inv_perm_out]
where R = Cayley-Neumann(skew(oft)) per 32x32 block, BD() is block-diagonal,
and P_in/P_out are the input/output feature permutations.

Device pipeline (replicated on all 8 cores; x sharded along tokens):
  1. Q_flat = vec^T @ E (E: host-built one-hot skew-scatter matrix)
  2. BD4 tiles of Q -> Cayley powers on PE -> R_left / R_right^T block-diag tiles
  3. H = BD(R_left)^T @ [W] ; rows scatter-stored by perm_out  (DRAM roundtrip 1)
  4. H2 reload -> PE transpose -> G2 = BD(R_right)^T^T... = BD_R @ H2^T ;
     rows scatter-stored by perm_in -> W_eff (f32r)           (DRAM roundtrip 2)
  5. GEMM: out[t,n] = sum_k xT[k,t] * W_eff[k,n] + b_eff[n]   (float32r matmuls)

Host does layout-only work: shard x along tokens, transpose per-core shard
(fp32 DMA transpose is unsupported on this stack), concat oft_L/oft_R, and
build integer index/one-hot constants from the permutation/index buffers.
"""

import numpy as np

IN_F = 2048
OUT_F = 2048
BS = 32
N_ELEM = BS * (BS - 1) // 2  # 496
N_BLOCKS = 128  # 64 left + 64 right
N_CORES = 8
TOKENS = 4 * 8192
TOKPC = TOKENS // N_CORES  # 4096
KB = IN_F // 128  # 16 k-blocks
NB = OUT_F // 128  # 16 n-blocks

_CACHE = {}


def _build(tokpc, use_f32r=True):
    import concourse.bass as bass
    import concourse.bacc as bacc
    import concourse.mybir as mybir
    import concourse.tile as tile
    from concourse.masks import make_identity

    dt = mybir.dt
    mmdt = dt.float32r if use_f32r else dt.float32

    def mm_in(ap):
        return ap.bitcast(dt.float32r) if use_f32r else ap

    SUP = 256  # token super-tile
    n_sup = tokpc // SUP
    MT = SUP // 128  # m-tiles per super

    nc = bacc.Bacc(None, target_bir_lowering=False, debug=False,
                   enable_asserts=False, num_devices=1)

    xt_in = nc.dram_tensor("xt", [IN_F, tokpc], dt.float32, kind="ExternalInput").ap()
    w_in = nc.dram_tensor("w", [OUT_F, IN_F], dt.float32, kind="ExternalInput").ap()
    b_in = nc.dram_tensor("b", [OUT_F, 1], dt.float32, kind="ExternalInput").ap()
    oft_in = nc.dram_tensor("oft", [N_BLOCKS, N_ELEM], dt.float32, kind="ExternalInput").ap()
    emat_in = nc.dram_tensor("emat", [N_ELEM, BS * BS], dt.float32, kind="ExternalInput").ap()
    pout_in = nc.dram_tensor("pout", [OUT_F, 1], dt.int32, kind="ExternalInput").ap()
    pin_in = nc.dram_tensor("pin", [IN_F, 1], dt.int32, kind="ExternalInput").ap()
    out_d = nc.dram_tensor("out", [tokpc, OUT_F], dt.float32, kind="ExternalOutput").ap()

    qflat_d = nc.dram_tensor("qflat_d", [N_BLOCKS, BS, BS], dt.float32).ap()
    h2_d = nc.dram_tensor("h2_d", [OUT_F, IN_F], dt.float32).ap()
    weff_d = nc.dram_tensor("weff_d", [IN_F, OUT_F],
                            dt.float32r if use_f32r else dt.float32).ap()
    b2_d = nc.dram_tensor("b2_d", [OUT_F, 1], dt.float32).ap()

    with tile.TileContext(nc) as tc:
        with tc.tile_pool(name="const", bufs=1) as const:
            ident = const.tile([128, 128], dt.float32)
            make_identity(nc, ident)

            # ---------------- Phase Q: Q_flat = vec^T @ E ----------------
            with tc.tile_pool(name="sbq", bufs=1) as sbq, \
                 tc.tile_pool(name="psq", bufs=1, space="PSUM") as psq:
                oft_t = sbq.tile([128, N_ELEM], dt.float32)
                nc.sync.dma_start(oft_t[:], oft_in[:])
                qps = psq.tile([128, BS * BS], dt.float32)
                CH = 124
                for c in range(4):
                    lo = c * CH
                    sz = min(CH, N_ELEM - lo)
                    tp = psq.tile([CH, 128], dt.float32, tag="tps")
                    nc.tensor.transpose(out=tp[:sz, :], in_=oft_t[:, lo:lo + sz],
                                        identity=ident[:])
                    vt = sbq.tile([CH, 128], dt.float32, tag="vt")
                    nc.any.tensor_copy(out=vt[:sz, :], in_=tp[:sz, :])
                    et = sbq.tile([CH, BS * BS], dt.float32, tag="et")
                    nc.sync.dma_start(et[:sz, :], emat_in[lo:lo + sz, :])
                    for nh in range(2):
                        nc.tensor.matmul(out=qps[:, nh * 512:(nh + 1) * 512],
                                         lhsT=vt[:sz, :],
                                         rhs=et[:sz, nh * 512:(nh + 1) * 512],
                                         start=(c == 0), stop=(c == 3))
                qsb = sbq.tile([128, BS * BS], dt.float32)
                nc.any.tensor_copy(out=qsb[:], in_=qps[:])
                nc.sync.dma_start(qflat_d[:].rearrange("p a b -> p (a b)"), qsb[:])

            # ---------------- Phase C: BD4 Q tiles + Cayley ----------------
            # r_tiles[g]: g<16 -> BD4(R_left[4g..4g+3]) ; g>=16 -> BD4(R_right^T) = BD4(R(-Q))
            with tc.tile_pool(name="rpool", bufs=32) as rpool, \
                 tc.tile_pool(name="rf32p", bufs=16) as rf32p:
                r_tiles = []
                rf_tiles = []
                with tc.tile_pool(name="sbc", bufs=2) as sbc, \
                     tc.tile_pool(name="bdqp", bufs=4) as bdqp, \
                     tc.tile_pool(name="psc", bufs=2, space="PSUM") as psc:
                    for g in range(32):
                        bdq = bdqp.tile([128, 128], dt.float32, tag="bdq")
                        nc.any.memset(bdq[:], 0.0)
                        for r in range(4):
                            nc.sync.dma_start(
                                bdq[r * BS:(r + 1) * BS, r * BS:(r + 1) * BS],
                                qflat_d[4 * g + r])
                        neg = sbc.tile([128, 128], dt.float32, tag="neg")
                        nc.vector.tensor_scalar_mul(out=neg[:], in0=bdq[:], scalar1=-1.0)
                        p2ps = psc.tile([128, 128], dt.float32, tag="p2ps")
                        nc.tensor.matmul(out=p2ps[:], lhsT=neg[:], rhs=bdq[:],
                                         start=True, stop=True)
                        p2 = sbc.tile([128, 128], dt.float32, tag="p2")
                        nc.any.tensor_copy(out=p2[:], in_=p2ps[:])
                        p3ps = psc.tile([128, 128], dt.float32, tag="p3ps")
                        nc.tensor.matmul(out=p3ps[:], lhsT=p2[:], rhs=bdq[:],
                                         start=True, stop=True)
                        p3 = sbc.tile([128, 128], dt.float32, tag="p3")
                        nc.any.tensor_copy(out=p3[:], in_=p3ps[:])
                        negp3 = sbc.tile([128, 128], dt.float32, tag="negp3")
                        nc.vector.tensor_scalar_mul(out=negp3[:], in0=p3[:], scalar1=-1.0)
                        p4ps = psc.tile([128, 128], dt.float32, tag="p4ps")
                        nc.tensor.matmul(out=p4ps[:], lhsT=negp3[:], rhs=bdq[:],
                                         start=True, stop=True)
                        # R = I + 2*(Q + P2 + P3 + P4)       (g < 16)
                        # R = I + 2*(-Q + P2 - P3 + P4)      (g >= 16, i.e. R(-Q))
                        t1 = sbc.tile([128, 128], dt.float32, tag="t1")
                        nc.vector.tensor_add(out=t1[:], in0=p2[:], in1=p4ps[:])
                        t2 = sbc.tile([128, 128], dt.float32, tag="t2")
                        nc.vector.tensor_add(out=t2[:], in0=bdq[:], in1=p3[:])
                        t3 = sbc.tile([128, 128], dt.float32, tag="t3")
                        op = mybir.AluOpType.add if g < 16 else mybir.AluOpType.subtract
                        nc.vector.tensor_tensor(out=t3[:], in0=t1[:], in1=t2[:], op=op)
                        nc.vector.tensor_scalar_mul(out=t3[:], in0=t3[:], scalar1=2.0)
                        rg = rpool.tile([128, 128], mmdt, tag="rg")
                        nc.vector.tensor_add(out=rg[:], in0=t3[:], in1=ident[:])
                        r_tiles.append(rg)
                        if g < 16:
                            rf = rf32p.tile([128, 128], dt.float32, tag="rf")
                            nc.vector.tensor_add(out=rf[:], in0=t3[:], in1=ident[:])
                            rf_tiles.append(rf)

                # ---------------- Phase B: b_rot + scatter ----------------
                with tc.tile_pool(name="sbb", bufs=1) as sbb, \
                     tc.tile_pool(name="psb", bufs=1, space="PSUM") as psb:
                    b_sb = sbb.tile([128, NB], dt.float32)
                    nc.sync.dma_start(
                        b_sb[:], b_in[:].rearrange("(g p) one -> p (g one)", p=128))
                    brot = sbb.tile([128, NB], dt.float32)
                    for g in range(NB):
                        bps = psb.tile([128, 1], dt.float32, tag="bps")
                        nc.tensor.matmul(out=bps[:], lhsT=rf_tiles[g][:],
                                         rhs=b_sb[:, g:g + 1], start=True, stop=True)
                        nc.any.tensor_copy(out=brot[:, g:g + 1], in_=bps[:])
                    for g in range(NB):
                        pidx = sbb.tile([128, 1], dt.int32, tag="pidx")
                        nc.sync.dma_start(pidx[:, :1], pout_in[g * 128:(g + 1) * 128, :])
                        nc.gpsimd.indirect_dma_start(
                            out=b2_d[:], out_offset=bass.IndirectOffsetOnAxis(
                                ap=pidx[:, :1], axis=0),
                            in_=brot[:, g:g + 1], in_offset=None)

                # ---------------- Phase H: H = BD_L^T @ W, scatter by perm_out ----
                with tc.tile_pool(name="sbh", bufs=3) as sbh, \
                     tc.tile_pool(name="psh", bufs=1, space="PSUM") as psh:
                    for g in range(NB):
                        wt = sbh.tile([128, IN_F], mmdt, tag="wt")
                        nc.sync.dma_start(wt[:], mm_in(w_in[g * 128:(g + 1) * 128, :]))
                        hps = psh.tile([128, IN_F], dt.float32, tag="hps")
                        for n in range(IN_F // 512):
                            nc.tensor.matmul(out=hps[:, n * 512:(n + 1) * 512],
                                             lhsT=r_tiles[g][:],
                                             rhs=wt[:, n * 512:(n + 1) * 512],
                                             start=True, stop=True)
                        hsb = sbh.tile([128, IN_F], dt.float32, tag="hsb")
                        nc.any.tensor_copy(out=hsb[:], in_=hps[:])
                        pidx = sbh.tile([128, 1], dt.int32, tag="hpidx")
                        nc.sync.dma_start(pidx[:, :1], pout_in[g * 128:(g + 1) * 128, :])
                        nc.gpsimd.indirect_dma_start(
                            out=h2_d[:], out_offset=bass.IndirectOffsetOnAxis(
                                ap=pidx[:, :1], axis=0),
                            in_=hsb[:], in_offset=None)

                # ------- Phase T: H2 reload, PE-transpose, G2 = BD_R @ H2^T -------
                with tc.tile_pool(name="h2tp", bufs=KB) as h2tp, \
                     tc.tile_pool(name="sbt", bufs=3) as sbt, \
                     tc.tile_pool(name="pst", bufs=4, space="PSUM") as pst, \
                     tc.tile_pool(name="psg", bufs=1, space="PSUM") as psg:
                    h2t = []
                    for _i in range(KB):
                        h2t_i = h2tp.tile([128, OUT_F], mmdt, tag="h2t", name=f"h2t_{_i}")
                        h2t.append(h2t_i)
                    for g in range(NB):
                        h2row = sbt.tile([128, IN_F], dt.float32, tag="h2row")
                        nc.sync.dma_start(h2row[:], h2_d[g * 128:(g + 1) * 128, :])
                        for i in range(KB):
                            tp = pst.tile([128, 128], dt.float32, tag="ttp")
                            nc.tensor.transpose(out=tp[:], in_=h2row[:, i * 128:(i + 1) * 128],
                                                identity=ident[:])
                            nc.any.tensor_copy(out=h2t[i][:, g * 128:(g + 1) * 128],
                                               in_=tp[:])
                    for i in range(KB):
                        gps = psg.tile([128, OUT_F], dt.float32, tag="gps")
                        for n in range(OUT_F // 512):
                            nc.tensor.matmul(out=gps[:, n * 512:(n + 1) * 512],
                                             lhsT=r_tiles[NB + i][:],
                                             rhs=h2t[i][:, n * 512:(n + 1) * 512],
                                             start=True, stop=True)
                        gsb = sbt.tile([128, OUT_F],
                                       dt.float32r if use_f32r else dt.float32,
                                       tag="gsb")
                        nc.any.tensor_copy(out=gsb[:], in_=gps[:])
                        pidx = sbt.tile([128, 1], dt.int32, tag="gpidx")
                        nc.sync.dma_start(pidx[:, :1], pin_in[i * 128:(i + 1) * 128, :])
                        nc.gpsimd.indirect_dma_start(
                            out=weff_d[:], out_offset=bass.IndirectOffsetOnAxis(
                                ap=pidx[:, :1], axis=0),
                            in_=gsb[:], in_offset=None)

            # ---------------- Phase G: the main GEMM ----------------
            with tc.tile_pool(name="biasp", bufs=1) as biasp:
                # bias broadcast to all partitions via rank-1 matmul
                with tc.tile_pool(name="sbias", bufs=1) as sbias, \
                     tc.tile_pool(name="psbias", bufs=1, space="PSUM") as psbias:
                    b2row = sbias.tile([1, OUT_F], dt.float32)
                    nc.sync.dma_start(b2row[:1, :], b2_d[:].rearrange("a b -> b a"))
                    ones = sbias.tile([1, 128], dt.float32)
                    nc.any.memset(ones[:], 1.0)
                    bbps = psbias.tile([128, OUT_F], dt.float32)
                    for n in range(OUT_F // 512):
                        nc.tensor.matmul(out=bbps[:, n * 512:(n + 1) * 512],
                                         lhsT=ones[:1, :],
                                         rhs=b2row[:1, n * 512:(n + 1) * 512],
                                         start=True, stop=True)
                    bias_sb = biasp.tile([128, OUT_F], dt.float32)
                    nc.any.tensor_copy(out=bias_sb[:], in_=bbps[:])

                with tc.tile_pool(name="wfp", bufs=KB) as wfp, \
                     tc.tile_pool(name="sbg", bufs=2) as sbg, \
                     tc.tile_pool(name="osbp", bufs=2) as osbp, \
                     tc.tile_pool(name="psgm", bufs=2, space="PSUM") as psgm:
                    weff = []
                    for _k in range(KB):
                        weff_k = wfp.tile([128, OUT_F], mmdt, tag="weff", name=f"weff_{_k}")
                        weff.append(weff_k)
                    for k in range(KB):
                        nc.sync.dma_start(weff[k][:], weff_d[k * 128:(k + 1) * 128, :])

                    for s in range(n_sup):
                        xts = sbg.tile([128, KB, SUP], mmdt, tag="xts")
                        for k in range(KB):
                            nc.sync.dma_start(
                                xts[:, k, :],
                                mm_in(xt_in[k * 128:(k + 1) * 128, s * SUP:(s + 1) * SUP]))
                        for mt in range(MT):
                            gps = psgm.tile([128, OUT_F], dt.float32, tag="gemmps")
                            for k in range(KB):
                                for n in range(OUT_F // 512):
                                    nc.tensor.matmul(
                                        out=gps[:, n * 512:(n + 1) * 512],
                                        lhsT=xts[:, k, mt * 128:(mt + 1) * 128],
                                        rhs=weff[k][:, n * 512:(n + 1) * 512],
                                        start=(k == 0), stop=(k == KB - 1))
                            osb = osbp.tile([128, OUT_F], dt.float32, tag="osb")
                            nc.vector.tensor_add(out=osb[:], in0=gps[:], in1=bias_sb[:])
                            row0 = s * SUP + mt * 128
                            nc.sync.dma_start(out_d[row0:row0 + 128, :], osb[:])

    nc.compile()
    return nc


def _host_prep(inputs):
    rows = np.asarray(inputs["rows"]).astype(np.int64)
    cols = np.asarray(inputs["cols"]).astype(np.int64)
    emat = np.zeros((N_ELEM, BS * BS), dtype=np.float32)
    e_idx = np.arange(N_ELEM)
    emat[e_idx, rows * BS + cols] = 1.0
    emat[e_idx, cols * BS + rows] = -1.0
    oft = np.concatenate([np.asarray(inputs["oft_L"], dtype=np.float32),
                          np.asarray(inputs["oft_R"], dtype=np.float32)], axis=0)
    pout = np.asarray(inputs["perm_out"]).astype(np.int32).reshape(OUT_F, 1)
    pin = np.asarray(inputs["perm_in"]).astype(np.int32).reshape(IN_F, 1)
    w = np.ascontiguousarray(np.asarray(inputs["W"], dtype=np.float32))
    b = np.asarray(inputs["b"], dtype=np.float32).reshape(OUT_F, 1)
    return emat, oft, pout, pin, w, b


def kernel(**inputs):
    from concourse.bass_utils import run_bass_kernel_spmd

    key = ("full", TOKPC)
    if key not in _CACHE:
        _CACHE[key] = _build(TOKPC)
    nc = _CACHE[key]

    emat, oft, pout, pin, w, b = _host_prep(inputs)
    x = np.asarray(inputs["x"], dtype=np.float32).reshape(TOKENS, IN_F)

    in_maps = []
    for c in range(N_CORES):
        shard = x[c * TOKPC:(c + 1) * TOKPC]
        xt = np.ascontiguousarray(shard.T)
        in_maps.append({"xt": xt, "w": w, "b": b, "oft": oft, "emat": emat,
                        "pout": pout, "pin": pin})

    res = run_bass_kernel_spmd(nc, in_maps, core_ids=list(range(N_CORES)))
    out = np.concatenate([res.results[c]["out"] for c in range(N_CORES)], axis=0)
    return out.reshape(4, 8192, OUT_F)
